# revision 8
# baseline (speedup 1.0000x reference)
"""Chamfer loss kernel for 8 Trainium2 NeuronCores.

Strategy
--------
nd2[i,j] = -(||x_i||^2 + ||y_j||^2 - 2 x_i . y_j)  (negated squared distance)
is computed as a K=5 augmented matmul on the TensorEngine:
    lhsT col i = [||x_i||^2, 1, -2x_i1, -2x_i2, -2x_i3]
    rhs  col j = -[1, ||y_j||^2,  y_j1,  y_j2,  y_j3]
(The negation turns both chamfer min-reductions into max-reductions.)
Sharding: core c handles batch b=c//2, x-half h=c%2 (2048 x-points vs all
4096 y-points -> 8.4M pairs per core).

Per PSUM tile [128 x, 512 y]:
  - ACT copies PSUM -> SBUF fp16 (s16)
  - DVE: acc2_n = max(acc2_n, s16)      (per-y running max = -min2)
  - DVE: m1tmp[:, i, n] = rowmax(s16)   (per-x per-tile max)
Finals: m1 = rowmax(m1tmp) in one reduce; acc2 streamed out per y-chunk;
host does the 128-way partition max, negation, sqrt(eps+d2) and means.
"""

import numpy as np

import concourse.bacc as bacc
import concourse.bass as bass
import concourse.mybir as mybir
import concourse.tile as tile
from concourse import bass_utils

F16 = mybir.dt.float16
F32 = mybir.dt.float32
MAX_OP = mybir.AluOpType.max
AXIS_X = mybir.AxisListType.X

EPS = 1e-6
N_CORES = 8


def build_kernel(npx=2048, npy=4096):
    """Emit the per-core program (identical on all cores)."""
    nxc = npx // 128  # x-chunks (output partition tiles)
    nyc = npy // 512  # y-chunks (psum free tiles)
    nc = bacc.Bacc("TRN2", target_bir_lowering=False, debug=False,
                   num_devices=N_CORES)
    # xa and ya are packed in one tensor so every PE instruction sits behind
    # a single DMA semaphore (PE LDWEIGHTS supports only one sync wait).
    xya = nc.dram_tensor("xya", [5, npx + npy], F32, kind="ExternalInput").ap()
    o1 = nc.dram_tensor("o1", [128, nxc], F32, kind="ExternalOutput").ap()
    o2 = nc.dram_tensor("o2", [128, npy], F16, kind="ExternalOutput").ap()

    with tile.TileContext(nc) as tc:
        with (
            tc.tile_pool(name="consts", bufs=1) as consts,
            tc.tile_pool(name="work", bufs=6) as work,
            tc.tile_pool(name="mm_psum", bufs=4, space="PSUM") as mm_psum,
        ):
            xya_sb = consts.tile([5, npx + npy], F32)
            nc.gpsimd.dma_start(out=xya_sb[:], in_=xya)
            xa_sb = xya_sb[:, :npx]
            ya_sb = xya_sb[:, npx:]

            m1tmp = consts.tile([128, nxc * nyc], F32)
            m1 = consts.tile([128, nxc], F32)
            acc2 = consts.tile([128, npy], F16)

            for n in range(nyc):
                ya_n = ya_sb[:, n * 512:(n + 1) * 512]
                acc2_n = acc2[:, n * 512:(n + 1) * 512]
                for i in range(nxc):
                    ps = mm_psum.tile([128, 512], F32, tag="mm")
                    nc.tensor.matmul(
                        ps[:],
                        lhsT=xa_sb[:, i * 128:(i + 1) * 128],
                        rhs=ya_n,
                        start=True, stop=True,
                    )
                    s16 = work.tile([128, 512], F16, tag="s16")
                    nc.scalar.copy(out=s16[:], in_=ps[:])
                    # min2 (negated -> max) accumulate over x-chunks
                    if i == 0:
                        nc.vector.tensor_copy(out=acc2_n, in_=s16[:])
                    else:
                        nc.vector.tensor_tensor(
                            out=acc2_n, in0=acc2_n, in1=s16[:], op=MAX_OP)
                    # min1 (negated -> max) per-tile row max
                    slot = i * nyc + n
                    nc.vector.tensor_reduce(
                        out=m1tmp[:, slot:slot + 1], in_=s16[:],
                        axis=AXIS_X, op=MAX_OP)
                # acc2_n is final for this y-chunk: stream it out now so the
                # DMA overlaps the next chunk's compute (host does the
                # 128-way partition max).
                nc.sync.dma_start(
                    out=o2[:, n * 512:(n + 1) * 512], in_=acc2_n)

            nc.vector.tensor_reduce(
                out=m1[:], in_=m1tmp[:].rearrange("p (i n) -> p i n", n=nyc),
                axis=AXIS_X, op=MAX_OP)
            nc.sync.dma_start(out=o1, in_=m1[:])
    nc.compile()
    return nc


def _augment(X, Y):
    """X: [nx,3], Y: [ny,3] -> packed [5, nx+ny] float32 (y side negated)."""
    nx, ny = X.shape[0], Y.shape[0]
    xya = np.empty((5, nx + ny), np.float32)
    xya[0, :nx] = (X * X).sum(-1)
    xya[1, :nx] = 1.0
    xya[2:, :nx] = -2.0 * X.T
    xya[0, nx:] = -1.0
    xya[1, nx:] = -(Y * Y).sum(-1)
    xya[2:, nx:] = -Y.T
    return xya


def run_cores(x, y, trace=False):
    """Run the 8-core SPMD kernel; returns BassKernelResults."""
    bs, npts, _ = x.shape
    half = npts // 2
    nc = build_kernel(npx=half, npy=npts)
    in_maps = []
    for c in range(N_CORES):
        b, h = divmod(c, 2)
        X = x[b, h * half:(h + 1) * half]
        Y = y[b]
        in_maps.append({"xya": _augment(X, Y)})
    res = bass_utils.run_bass_kernel_spmd(
        nc, in_maps, core_ids=list(range(N_CORES)), trace=trace)
    return res


def _combine(res, bs):
    # outputs hold NEGATED squared distances (maxima); negate back.
    m1 = [-res.results[c]["o1"].T.reshape(-1).astype(np.float64)
          for c in range(N_CORES)]
    m2 = [-res.results[c]["o2"].astype(np.float32).max(axis=0)
          for c in range(N_CORES)]
    tot1 = 0.0
    tot2 = 0.0
    for b in range(bs):
        d1 = np.concatenate([m1[2 * b], m1[2 * b + 1]])
        d2 = np.minimum(m2[2 * b], m2[2 * b + 1]).astype(np.float64)
        tot1 += np.sqrt(EPS + d1).mean()
        tot2 += np.sqrt(EPS + d2).mean()
    return np.float32((tot1 + tot2) / bs)


def kernel(x, y):
    x = np.asarray(x, dtype=np.float32)
    y = np.asarray(y, dtype=np.float32)
    res = run_cores(x, y)
    return _combine(res, x.shape[0])


# revision 9
# speedup vs baseline: 2.1040x; 2.1040x over previous
"""Chamfer loss kernel for 8 Trainium2 NeuronCores.

Strategy
--------
nd2[i,j] = -(||x_i||^2 + ||y_j||^2 - 2 x_i . y_j)  (negated squared distance)
is computed as an augmented matmul on the TensorEngine:
    A col i = [||x_i||^2, 1, -2x_i1, -2x_i2, -2x_i3]
    B col j = -[1, ||y_j||^2,  y_j1,  y_j2,  y_j3]
(The negation turns both chamfer min-reductions into max-reductions.)
fp32 matmuls are 4-6x slower on the PE, so A and B are split hi/mid/lo into
bf16 (A = Ah+Am+Al) and the products are compensated in one K=30 bf16 matmul:
    A.B ~ AhBh + AhBm + AmBh + AhBl + AlBh + AmBm   (error ~1e-6 absolute)
Sharding: core c handles batch b=c//2, x-half h=c%2 (2048 x-points vs all
4096 y-points -> 8.4M pairs per core).

Per PSUM tile [128 x, 512 y]:
  - ACT copies PSUM -> SBUF fp16 (s16)
  - DVE: acc2_n = max(acc2_n, s16)      (per-y running max = -min2)
  - DVE: m1tmp[:, i, n] = rowmax(s16)   (per-x per-tile max)
Finals: m1 = rowmax(m1tmp) in one reduce; acc2 streamed out per y-chunk;
host does the 128-way partition max, negation, sqrt(eps+d2) and means.
"""

import numpy as np

import concourse.bacc as bacc
import concourse.bass as bass
import concourse.mybir as mybir
import concourse.tile as tile
from concourse import bass_utils

F16 = mybir.dt.float16
F32 = mybir.dt.float32
BF16 = mybir.dt.bfloat16
MAX_OP = mybir.AluOpType.max
AXIS_X = mybir.AxisListType.X

EPS = 1e-6
N_CORES = 8


def build_kernel(npx=2048, npy=4096):
    """Emit the per-core program (identical on all cores)."""
    nxc = npx // 128  # x-chunks (output partition tiles)
    nyc = npy // 512  # y-chunks (psum free tiles)
    nc = bacc.Bacc("TRN2", target_bir_lowering=False, debug=False,
                   num_devices=N_CORES)
    # xa and ya are packed in one tensor so every PE instruction sits behind
    # a single DMA semaphore (PE LDWEIGHTS supports only one sync wait).
    xya = nc.dram_tensor("xya", [30, npx + npy], BF16, kind="ExternalInput").ap()
    o1 = nc.dram_tensor("o1", [128, nxc], F32, kind="ExternalOutput").ap()
    o2 = nc.dram_tensor("o2", [128, npy], F16, kind="ExternalOutput").ap()

    with tile.TileContext(nc) as tc:
        with (
            tc.tile_pool(name="consts", bufs=1) as consts,
            tc.tile_pool(name="work", bufs=6) as work,
            tc.tile_pool(name="mm_psum", bufs=4, space="PSUM") as mm_psum,
        ):
            xya_sb = consts.tile([30, npx + npy], BF16)
            nc.gpsimd.dma_start(out=xya_sb[:], in_=xya)
            xa_sb = xya_sb[:, :npx]
            ya_sb = xya_sb[:, npx:]

            m1tmp = consts.tile([128, nxc * nyc], F32)
            m1 = consts.tile([128, nxc], F32)
            acc2 = consts.tile([128, npy], F16)

            for n in range(nyc):
                ya_n = ya_sb[:, n * 512:(n + 1) * 512]
                acc2_n = acc2[:, n * 512:(n + 1) * 512]
                for i in range(nxc):
                    ps = mm_psum.tile([128, 512], F32, tag="mm")
                    nc.tensor.matmul(
                        ps[:],
                        lhsT=xa_sb[:, i * 128:(i + 1) * 128],
                        rhs=ya_n,
                        start=True, stop=True,
                    )
                    s16 = work.tile([128, 512], F16, tag="s16")
                    nc.scalar.copy(out=s16[:], in_=ps[:])
                    # min2 (negated -> max) accumulate over x-chunks
                    if i == 0:
                        nc.vector.tensor_copy(out=acc2_n, in_=s16[:])
                    else:
                        nc.vector.tensor_tensor(
                            out=acc2_n, in0=acc2_n, in1=s16[:], op=MAX_OP)
                    # min1 (negated -> max) per-tile row max
                    slot = i * nyc + n
                    nc.vector.tensor_reduce(
                        out=m1tmp[:, slot:slot + 1], in_=s16[:],
                        axis=AXIS_X, op=MAX_OP)
                # acc2_n is final for this y-chunk: stream it out now so the
                # DMA overlaps the next chunk's compute (host does the
                # 128-way partition max).
                nc.sync.dma_start(
                    out=o2[:, n * 512:(n + 1) * 512], in_=acc2_n)

            nc.vector.tensor_reduce(
                out=m1[:], in_=m1tmp[:].rearrange("p (i n) -> p i n", n=nyc),
                axis=AXIS_X, op=MAX_OP)
            nc.sync.dma_start(out=o1, in_=m1[:])
    nc.compile()
    return nc


def _augment(X, Y):
    """X: [nx,3], Y: [ny,3] -> packed [30, nx+ny] bf16 (y side negated).

    Rows are a compensated bf16 hi/mid/lo split of the augmented 5-vectors
    A (x side) and B (y side), paired so that the K=30 contraction computes
    AhBh + AhBm + AmBh + AhBl + AlBh + AmBm ~= A.B to ~1e-6 absolute.
    """
    import ml_dtypes
    bf16 = ml_dtypes.bfloat16
    nx, ny = X.shape[0], Y.shape[0]
    A = np.empty((5, nx), np.float32)
    A[0] = (X * X).sum(-1)
    A[1] = 1.0
    A[2:] = -2.0 * X.T
    B = np.empty((5, ny), np.float32)
    B[0] = -1.0
    B[1] = -(Y * Y).sum(-1)
    B[2:] = -Y.T

    def split3(M):
        h = M.astype(bf16)
        r = M - h.astype(np.float32)
        m = r.astype(bf16)
        l = (r - m.astype(np.float32)).astype(bf16)
        return h, m, l

    Ah, Am, Al = split3(A)
    Bh, Bm, Bl = split3(B)
    xya = np.empty((30, nx + ny), bf16)
    for g, (a, b) in enumerate([(Ah, Bh), (Ah, Bm), (Am, Bh),
                                (Ah, Bl), (Al, Bh), (Am, Bm)]):
        xya[5 * g:5 * g + 5, :nx] = a
        xya[5 * g:5 * g + 5, nx:] = b
    return xya


def run_cores(x, y, trace=False):
    """Run the 8-core SPMD kernel; returns BassKernelResults."""
    bs, npts, _ = x.shape
    half = npts // 2
    nc = build_kernel(npx=half, npy=npts)
    in_maps = []
    for c in range(N_CORES):
        b, h = divmod(c, 2)
        X = x[b, h * half:(h + 1) * half]
        Y = y[b]
        in_maps.append({"xya": _augment(X, Y)})
    res = bass_utils.run_bass_kernel_spmd(
        nc, in_maps, core_ids=list(range(N_CORES)), trace=trace)
    return res


def _combine(res, bs):
    # outputs hold NEGATED squared distances (maxima); negate back.
    m1 = [-res.results[c]["o1"].T.reshape(-1).astype(np.float64)
          for c in range(N_CORES)]
    m2 = [-res.results[c]["o2"].astype(np.float32).max(axis=0)
          for c in range(N_CORES)]
    tot1 = 0.0
    tot2 = 0.0
    for b in range(bs):
        d1 = np.concatenate([m1[2 * b], m1[2 * b + 1]])
        d2 = np.minimum(m2[2 * b], m2[2 * b + 1]).astype(np.float64)
        tot1 += np.sqrt(EPS + d1).mean()
        tot2 += np.sqrt(EPS + d2).mean()
    return np.float32((tot1 + tot2) / bs)


def kernel(x, y):
    x = np.asarray(x, dtype=np.float32)
    y = np.asarray(y, dtype=np.float32)
    res = run_cores(x, y)
    return _combine(res, x.shape[0])


# revision 10
# speedup vs baseline: 2.6731x; 1.2705x over previous
"""Chamfer loss kernel for 8 Trainium2 NeuronCores.

Strategy
--------
nd2[i,j] = -(||x_i||^2 + ||y_j||^2 - 2 x_i . y_j)  (negated squared distance)
is computed as an augmented matmul on the TensorEngine:
    A col i = [||x_i||^2, 1, -2x_i1, -2x_i2, -2x_i3]
    B col j = -[1, ||y_j||^2,  y_j1,  y_j2,  y_j3]
(The negation turns both chamfer min-reductions into max-reductions.)
fp32 matmuls are 4-6x slower on the PE, so A and B are split hi/mid/lo into
bf16 (A = Ah+Am+Al) and the products are compensated in one K=30 bf16 matmul:
    A.B ~ AhBh + AhBm + AmBh + AhBl + AlBh + AmBm   (error ~1e-6 absolute)
Sharding: core c handles batch b=c//2, x-half h=c%2 (2048 x-points vs all
4096 y-points -> 8.4M pairs per core).

Loop: x-chunk outer (128 x-points), y in quads of QW=2048 (4 PSUM banks):
  PE:  4 matmuls N=512 -> psum quad [128, 2048] fp32
  ACT: one copy psum -> s16 fp16 [128, 2048]
  DVE: min2: acc2[:, quad] = max(acc2, s16)      (fp16 2x tensor_tensor)
       min1: m1acc = max(m1acc, s16[:, t*512:])  (4 folds, fp16 2x)
  per x-chunk final: m1[:, i] = rowmax(m1acc)    (one 1x reduce, overlapped)
Host: augmentation prep (O(N)), 128-way partition max of acc2, negation,
sqrt(eps+d2), means.
"""

import numpy as np

import concourse.bacc as bacc
import concourse.mybir as mybir
import concourse.tile as tile
from concourse import bass_utils

F16 = mybir.dt.float16
F32 = mybir.dt.float32
BF16 = mybir.dt.bfloat16
MAX_OP = mybir.AluOpType.max
AXIS_X = mybir.AxisListType.X

EPS = 1e-6
N_CORES = 8
MM_N = 512          # matmul free dim (one PSUM bank)


def build_kernel(npx=2048, npy=4096):
    """Emit the per-core program (identical on all cores)."""
    nxc = npx // 128            # x-chunks
    qw = min(2048, npy)         # y quad width (4 PSUM banks)
    nq = npy // qw              # quads per row
    mm_per_q = qw // MM_N
    nc = bacc.Bacc("TRN2", target_bir_lowering=False, debug=False,
                   num_devices=N_CORES)
    # xa and ya are packed in one tensor so every PE instruction sits behind
    # a single DMA semaphore (PE LDWEIGHTS supports only one sync wait).
    xya = nc.dram_tensor("xya", [30, npx + npy], BF16, kind="ExternalInput").ap()
    o1 = nc.dram_tensor("o1", [128, nxc], F32, kind="ExternalOutput").ap()
    o2 = nc.dram_tensor("o2", [128, npy], F16, kind="ExternalOutput").ap()

    with tile.TileContext(nc) as tc:
        with (
            tc.tile_pool(name="consts", bufs=1) as consts,
            tc.tile_pool(name="work", bufs=4) as work,
            tc.tile_pool(name="m1p", bufs=2) as m1p,
            tc.tile_pool(name="mm_psum", bufs=2, space="PSUM") as mm_psum,
        ):
            xya_sb = consts.tile([30, npx + npy], BF16)
            nc.gpsimd.dma_start(out=xya_sb[:], in_=xya)
            xa_sb = xya_sb[:, :npx]
            ya_sb = xya_sb[:, npx:]

            m1 = consts.tile([128, nxc], F32)
            acc2 = consts.tile([128, npy], F16)

            for i in range(nxc):
                lhsT = xa_sb[:, i * 128:(i + 1) * 128]
                m1acc = m1p.tile([128, MM_N], F16, tag="m1acc")
                first = True
                for q in range(nq):
                    ps = mm_psum.tile([128, qw], F32, tag="mm")
                    for s in range(mm_per_q):
                        nc.tensor.matmul(
                            ps[:, s * MM_N:(s + 1) * MM_N],
                            lhsT=lhsT,
                            rhs=ya_sb[:, q * qw + s * MM_N:
                                      q * qw + (s + 1) * MM_N],
                            start=True, stop=True,
                        )
                    s16 = work.tile([128, qw], F16, tag="s16")
                    nc.scalar.copy(out=s16[:], in_=ps[:])
                    # min2 (negated -> max) accumulate over x-chunks
                    acc2_q = acc2[:, q * qw:(q + 1) * qw]
                    if i == 0:
                        nc.vector.tensor_copy(out=acc2_q, in_=s16[:])
                    else:
                        nc.vector.tensor_tensor(
                            out=acc2_q, in0=acc2_q, in1=s16[:], op=MAX_OP)
                    # min1 (negated -> max): fold s16 into m1acc per MM_N slice
                    for t in range(qw // MM_N):
                        sl = s16[:, t * MM_N:(t + 1) * MM_N]
                        if first:
                            nc.vector.tensor_copy(out=m1acc[:], in_=sl)
                            first = False
                        else:
                            nc.vector.tensor_tensor(
                                out=m1acc[:], in0=m1acc[:], in1=sl, op=MAX_OP)
                # per-x-chunk min1 final (overlaps next chunk's work)
                nc.vector.tensor_reduce(
                    out=m1[:, i:i + 1], in_=m1acc[:], axis=AXIS_X, op=MAX_OP)

            for q in range(nq):
                nc.sync.dma_start(out=o2[:, q * qw:(q + 1) * qw],
                                  in_=acc2[:, q * qw:(q + 1) * qw])
            nc.sync.dma_start(out=o1, in_=m1[:])
    nc.compile()
    return nc


def _augment(X, Y):
    """X: [nx,3], Y: [ny,3] -> packed [30, nx+ny] bf16 (y side negated).

    Rows are a compensated bf16 hi/mid/lo split of the augmented 5-vectors
    A (x side) and B (y side), paired so that the K=30 contraction computes
    AhBh + AhBm + AmBh + AhBl + AlBh + AmBm ~= A.B to ~1e-6 absolute.
    """
    import ml_dtypes
    bf16 = ml_dtypes.bfloat16
    nx, ny = X.shape[0], Y.shape[0]
    A = np.empty((5, nx), np.float32)
    A[0] = (X * X).sum(-1)
    A[1] = 1.0
    A[2:] = -2.0 * X.T
    B = np.empty((5, ny), np.float32)
    B[0] = -1.0
    B[1] = -(Y * Y).sum(-1)
    B[2:] = -Y.T

    def split3(M):
        h = M.astype(bf16)
        r = M - h.astype(np.float32)
        m = r.astype(bf16)
        l = (r - m.astype(np.float32)).astype(bf16)
        return h, m, l

    Ah, Am, Al = split3(A)
    Bh, Bm, Bl = split3(B)
    xya = np.empty((30, nx + ny), bf16)
    for g, (a, b) in enumerate([(Ah, Bh), (Ah, Bm), (Am, Bh),
                                (Ah, Bl), (Al, Bh), (Am, Bm)]):
        xya[5 * g:5 * g + 5, :nx] = a
        xya[5 * g:5 * g + 5, nx:] = b
    return xya


def run_cores(x, y, trace=False):
    """Run the 8-core SPMD kernel; returns BassKernelResults."""
    bs, npts, _ = x.shape
    half = npts // 2
    nc = build_kernel(npx=half, npy=npts)
    in_maps = []
    for c in range(N_CORES):
        b, h = divmod(c, 2)
        X = x[b, h * half:(h + 1) * half]
        Y = y[b]
        in_maps.append({"xya": _augment(X, Y)})
    res = bass_utils.run_bass_kernel_spmd(
        nc, in_maps, core_ids=list(range(N_CORES)), trace=trace)
    return res


def _combine(res, bs):
    # outputs hold NEGATED squared distances (maxima); negate back.
    m1 = [-res.results[c]["o1"].T.reshape(-1).astype(np.float64)
          for c in range(N_CORES)]
    m2 = [-res.results[c]["o2"].astype(np.float32).max(axis=0)
          for c in range(N_CORES)]
    tot1 = 0.0
    tot2 = 0.0
    for b in range(bs):
        d1 = np.concatenate([m1[2 * b], m1[2 * b + 1]])
        d2 = np.minimum(m2[2 * b], m2[2 * b + 1]).astype(np.float64)
        tot1 += np.sqrt(EPS + d1).mean()
        tot2 += np.sqrt(EPS + d2).mean()
    return np.float32((tot1 + tot2) / bs)


def kernel(x, y):
    x = np.asarray(x, dtype=np.float32)
    y = np.asarray(y, dtype=np.float32)
    res = run_cores(x, y)
    return _combine(res, x.shape[0])


# revision 12
# speedup vs baseline: 3.1188x; 1.1667x over previous
"""Chamfer loss kernel for 8 Trainium2 NeuronCores.

Strategy
--------
nd2[i,j] = -(||x_i||^2 + ||y_j||^2 - 2 x_i . y_j)  (negated squared distance)
is computed as an augmented matmul on the TensorEngine:
    A col i = [||x_i||^2, 1, -2x_i1, -2x_i2, -2x_i3]
    B col j = -[1, ||y_j||^2,  y_j1,  y_j2,  y_j3]
(The negation turns both chamfer min-reductions into max-reductions.)
fp32 matmuls are 4-6x slower on the PE, so A and B are split hi/mid/lo into
bf16 (A = Ah+Am+Al) and the products are compensated in one K=30 bf16 matmul:
    A.B ~ AhBh + AhBm + AmBh + AhBl + AlBh + AmBm   (error ~1e-6 absolute)
Sharding: core c handles batch b=c//2, x-half h=c%2 (2048 x-points vs all
4096 y-points -> 8.4M pairs per core).

Loop: x-chunk outer (128 x-points), y in quads of QW=2048 (4 PSUM banks):
  PE:  4 matmuls N=512 -> psum quad [128, 2048] fp32
  ACT: one copy psum -> s16 fp16 [128, 2048]
  DVE: min2: acc2[:, quad] = max(acc2, s16)      (fp16 2x tensor_tensor)
       min1: m1acc = max(m1acc, s16[:, t*512:])  (4 folds, fp16 2x)
  per x-chunk final: m1[:, i] = rowmax(m1acc)    (one 1x reduce, overlapped)
Host: augmentation prep (O(N)), 128-way partition max of acc2, negation,
sqrt(eps+d2), means.
"""

import numpy as np

import concourse.bacc as bacc
import concourse.mybir as mybir
import concourse.tile as tile
from concourse import bass_utils

F16 = mybir.dt.float16
F32 = mybir.dt.float32
BF16 = mybir.dt.bfloat16
MAX_OP = mybir.AluOpType.max
AXIS_X = mybir.AxisListType.X

EPS = 1e-6
N_CORES = 8
MM_N = 512          # matmul free dim (one PSUM bank)
MW = 1024           # min1 accumulator width (host reduces the rest)


def build_kernel(npx=2048, npy=4096):
    """Emit the per-core program (identical on all cores)."""
    nxc = npx // 128            # x-chunks
    qw = min(2048, npy)         # y quad width (4 PSUM banks)
    nq = npy // qw              # quads per row
    mm_per_q = qw // MM_N
    nc = bacc.Bacc("TRN2", target_bir_lowering=False, debug=False,
                   num_devices=N_CORES)
    # xa and ya are packed in one tensor so every PE instruction sits behind
    # a single DMA semaphore (PE LDWEIGHTS supports only one sync wait).
    xya = nc.dram_tensor("xya", [30, npx + npy], BF16, kind="ExternalInput").ap()
    o1 = nc.dram_tensor("o1", [128, nxc * MW], F16, kind="ExternalOutput").ap()
    o2 = nc.dram_tensor("o2", [128, npy], F16, kind="ExternalOutput").ap()

    with tile.TileContext(nc) as tc:
        with (
            tc.tile_pool(name="consts", bufs=1) as consts,
            tc.tile_pool(name="work", bufs=4) as work,
            tc.tile_pool(name="m1p", bufs=2) as m1p,
            tc.tile_pool(name="mm_psum", bufs=2, space="PSUM") as mm_psum,
        ):
            xya_sb = consts.tile([30, npx + npy], BF16)
            nc.gpsimd.dma_start(out=xya_sb[:], in_=xya)
            xa_sb = xya_sb[:, :npx]
            ya_sb = xya_sb[:, npx:]

            acc2 = consts.tile([128, npy], F16)

            for i in range(nxc):
                lhsT = xa_sb[:, i * 128:(i + 1) * 128]
                m1acc = m1p.tile([128, MW], F16, tag="m1acc")
                first = True
                for q in range(nq):
                    ps = mm_psum.tile([128, qw], F32, tag="mm")
                    for s in range(mm_per_q):
                        nc.tensor.matmul(
                            ps[:, s * MM_N:(s + 1) * MM_N],
                            lhsT=lhsT,
                            rhs=ya_sb[:, q * qw + s * MM_N:
                                      q * qw + (s + 1) * MM_N],
                            start=True, stop=True,
                        )
                    s16 = work.tile([128, qw], F16, tag="s16")
                    nc.scalar.copy(out=s16[:], in_=ps[:])
                    # min2 (negated -> max) accumulate over x-chunks
                    acc2_q = acc2[:, q * qw:(q + 1) * qw]
                    if i == 0:
                        nc.vector.tensor_copy(out=acc2_q, in_=s16[:])
                    else:
                        nc.vector.tensor_tensor(
                            out=acc2_q, in0=acc2_q, in1=s16[:], op=MAX_OP)
                    # min1 (negated -> max): fold s16 into m1acc per MW slice
                    for t in range(qw // MW):
                        sl = s16[:, t * MW:(t + 1) * MW]
                        if first:
                            nc.vector.tensor_copy(out=m1acc[:], in_=sl)
                            first = False
                        else:
                            nc.vector.tensor_tensor(
                                out=m1acc[:], in0=m1acc[:], in1=sl, op=MAX_OP)
                # stream the chunk's min1 fold buffer out; host does the
                # final MW-wide row max.
                nc.sync.dma_start(out=o1[:, i * MW:(i + 1) * MW], in_=m1acc[:])

            for q in range(nq):
                nc.sync.dma_start(out=o2[:, q * qw:(q + 1) * qw],
                                  in_=acc2[:, q * qw:(q + 1) * qw])
    nc.compile()
    return nc


def _augment(X, Y):
    """X: [nx,3], Y: [ny,3] -> packed [30, nx+ny] bf16 (y side negated).

    Rows are a compensated bf16 hi/mid/lo split of the augmented 5-vectors
    A (x side) and B (y side), paired so that the K=30 contraction computes
    AhBh + AhBm + AmBh + AhBl + AlBh + AmBm ~= A.B to ~1e-6 absolute.
    """
    import ml_dtypes
    bf16 = ml_dtypes.bfloat16
    nx, ny = X.shape[0], Y.shape[0]
    A = np.empty((5, nx), np.float32)
    A[0] = (X * X).sum(-1)
    A[1] = 1.0
    A[2:] = -2.0 * X.T
    B = np.empty((5, ny), np.float32)
    B[0] = -1.0
    B[1] = -(Y * Y).sum(-1)
    B[2:] = -Y.T

    def split3(M):
        h = M.astype(bf16)
        r = M - h.astype(np.float32)
        m = r.astype(bf16)
        l = (r - m.astype(np.float32)).astype(bf16)
        return h, m, l

    Ah, Am, Al = split3(A)
    Bh, Bm, Bl = split3(B)
    xya = np.empty((30, nx + ny), bf16)
    for g, (a, b) in enumerate([(Ah, Bh), (Ah, Bm), (Am, Bh),
                                (Ah, Bl), (Al, Bh), (Am, Bm)]):
        xya[5 * g:5 * g + 5, :nx] = a
        xya[5 * g:5 * g + 5, nx:] = b
    return xya


def run_cores(x, y, trace=False):
    """Run the 8-core SPMD kernel; returns BassKernelResults."""
    bs, npts, _ = x.shape
    half = npts // 2
    nc = build_kernel(npx=half, npy=npts)
    in_maps = []
    for c in range(N_CORES):
        b, h = divmod(c, 2)
        X = x[b, h * half:(h + 1) * half]
        Y = y[b]
        in_maps.append({"xya": _augment(X, Y)})
    res = bass_utils.run_bass_kernel_spmd(
        nc, in_maps, core_ids=list(range(N_CORES)), trace=trace)
    return res


def _combine(res, bs):
    # outputs hold NEGATED squared distances (maxima); negate back.
    def host_m1(o1b):
        nxc = o1b.shape[1] // MW
        v = o1b.astype(np.float32).reshape(128, nxc, MW).max(axis=2)
        return -v.T.reshape(-1).astype(np.float64)

    m1 = [host_m1(res.results[c]["o1"]) for c in range(N_CORES)]
    m2 = [-res.results[c]["o2"].astype(np.float32).max(axis=0)
          for c in range(N_CORES)]
    tot1 = 0.0
    tot2 = 0.0
    for b in range(bs):
        d1 = np.concatenate([m1[2 * b], m1[2 * b + 1]])
        d2 = np.minimum(m2[2 * b], m2[2 * b + 1]).astype(np.float64)
        tot1 += np.sqrt(EPS + d1).mean()
        tot2 += np.sqrt(EPS + d2).mean()
    return np.float32((tot1 + tot2) / bs)


def kernel(x, y):
    x = np.asarray(x, dtype=np.float32)
    y = np.asarray(y, dtype=np.float32)
    res = run_cores(x, y)
    return _combine(res, x.shape[0])
